# revision 10
# baseline (speedup 1.0000x reference)
"""ColBERT MaxSim kernel for 8 Trainium2 NeuronCores (Bass/Tile).

Strategy: data-parallel over the 256-doc batch (32 docs per core).

Host side:
  - compacts each doc's VALID tokens (d_mask is ~50% dense) to the
    front and pads to a per-quad budget with a COPY of the doc's first
    valid token.  Duplicating a valid token leaves the per-(query,doc)
    max unchanged, so this is exactly equivalent to -inf masking.
  - VARIABLE per-quad budgets: each core holds 4 query batches x 8
    docs; each batch's docs are sorted by valid count, and quad g takes
    the rank-g doc of every batch (position d <-> batch d, so the
    kernel's doc->query mapping stays compile-time static).  Budget
    lt[g] = max valid count over quad-slot g across all cores (rounded
    to 16), so early quads are ~288 tokens and late ones ~240.
  - computes the query side on host in fp32 and folds it through W:
    qw = W.T @ l2norm(W @ q) [H, 128q], so the device computes raw
    scores DIRECTLY off the fp8 doc stream (contraction over H=768).
  - W and qw are pre-scaled by 8 so fp8(e4m3) entries land in the
    normal range; the scales cancel exactly in sim * rsqrt(|8Wd|^2).

Per core (32 docs = 8 quads):
  per pair of docs (DoubleRow fp8, K=256 per pass):
    pd[:, j] = W8.T @ dT[d]            [128dim, lt] f32 PSUM
    sq = pd^2   (one ACT square per pair, bf16 -> SBUF)
  per quad g (4 docs on col-groups cg = d%4; M=32 matmuls run
  CONCURRENTLY in distinct 32-col groups of the PE array):
    sim[32cg:+32, :] += qw8[:, c, 32cg].T @ dT[d]   (6 passes, fp8)
    ssq[32cg:+32, :]  = ones[:, :32].T @ sq[d]      (per-token sumsq)
    invb = rsqrt(ssq + eps)            (ACT)
    scaled = sim * invb                (DVE, bf16)
    maxcol[:, g] = max_tok(scaled)     (DVE reduce_max)
  out[4, 8] = blockones.T @ maxcol     (sum over 32 queries via matmul)

PE warm-up: the HAM clock gate keeps the PE at 1.2 GHz until ~3.4us of
sustained activity; dummy fp8 matmuls run while the first doc slab is
still in flight so the real work starts at 2.4 GHz.
"""

import numpy as np
import ml_dtypes

import concourse.bass as bass
import concourse.bacc as bacc
import concourse.mybir as mybir
import concourse.tile as tile
from concourse.bass_utils import run_bass_kernel_spmd

N_CORES = 8
H, HC, P = 768, 6, 128   # hidden dim, h-chunks, partitions
DIM = 128                # projection dim
DPC = 32                 # docs per core
QPC = 128                # query vectors per core (4 batches x 32)
PPQ = 8                  # passages per query
NQUAD = DPC // 4
BF16 = mybir.dt.bfloat16
FP8 = mybir.dt.float8e4
F32 = mybir.dt.float32
EPS2 = 1e-12
LT_MIN = 64              # floor on per-quad compacted token budget
W8SCALE = 8.0            # fp8 pre-scale on W / qw; cancels in normalization
N_WARMUP = 40            # dummy PE matmuls to lift the HAM clock gate

_LTS = (288,) * NQUAD
_NC_CACHE = {}


def _rsqrt_act(nc, out, in_, bias_ap):
    """out = 1/sqrt(in_ + bias). Emits the Rsqrt activation directly
    (bass's helper refuses it; the 40k-entry reciprocal_sqrt HW table is
    plenty accurate for this kernel's fp8-dominated error budget)."""
    eng = nc.scalar
    ins = [eng.lower_ap(in_), eng.lower_ap(bias_ap),
           mybir.ImmediateValue(dtype=mybir.dt.float32, value=1.0),
           mybir.ImmediateValue(dtype=mybir.dt.float32, value=0.0)]
    return eng.add_instruction(mybir.InstActivation(
        name=nc.get_next_instruction_name(),
        func=mybir.ActivationFunctionType.Rsqrt,
        ins=ins, outs=[eng.lower_ap(out)]))


def _build_nc(lts):
    nc = bacc.Bacc()
    dt_d = [nc.declare_dram_parameter(f"dt{g}", [P, 4, HC, lts[g]], FP8,
                                      isOutput=False) for g in range(NQUAD)]
    qw_d = nc.declare_dram_parameter("qw", [P, HC, QPC], FP8, isOutput=False)
    wt8_d = nc.declare_dram_parameter("wt8", [P, HC, DIM], FP8, isOutput=False)
    out_d = nc.declare_dram_parameter("out", [4, NQUAD], F32, isOutput=True)
    DR = mybir.MatmulPerfMode.DoubleRow

    with tile.TileContext(nc) as tc:
        with (
            tc.tile_pool(name="const", bufs=1) as const,
            tc.tile_pool(name="slab", bufs=4) as slabp,
            tc.tile_pool(name="work", bufs=2) as work,
            tc.tile_pool(name="psum", bufs=2, space=bass.MemorySpace.PSUM) as psum,
        ):
            # ---- input DMAs, ordered for earliest PE start ----
            wt8_s = const.tile([P, HC, DIM], FP8)
            nc.sync.dma_start(out=wt8_s, in_=wt8_d[:])
            slabs = {}
            slab0 = slabp.tile([P, 4, HC, lts[0]], FP8, tag="slab")
            slabs[0] = slab0
            nc.sync.dma_start(out=slab0[:, 0], in_=dt_d[0][:, 0])
            nc.sync.dma_start(out=slab0[:, 1], in_=dt_d[0][:, 1])
            qw_s = const.tile([P, HC, QPC], FP8)
            nc.sync.dma_start(out=qw_s, in_=qw_d[:])
            nc.sync.dma_start(out=slab0[:, 2], in_=dt_d[0][:, 2])
            nc.sync.dma_start(out=slab0[:, 3], in_=dt_d[0][:, 3])

            # ---- constants ----
            ones_raw = const.tile([P, 32], BF16)
            nc.vector.memset(ones_raw, 1.0)
            ones_s = const.tile([P, 32], BF16)     # all-ones lhsT
            nc.scalar.copy(ones_s, ones_raw)
            blk_raw = const.tile([P, 4], F32)      # block-diag ones: col b = 1
            nc.vector.memset(blk_raw, 0.0)         # on partitions 32b..32b+32
            for b in range(4):
                nc.vector.memset(blk_raw[32 * b:32 * b + 32, b:b + 1], 1.0)
            blockones = const.tile([P, 4], F32)
            nc.scalar.copy(blockones, blk_raw)
            eps_t = const.tile([P, 1], F32)        # rsqrt bias (l2norm eps^2)
            nc.vector.memset(eps_t, EPS2)
            maxcol = const.tile([P, NQUAD], F32)   # [4docs x 32q, quads]

            # ---- PE warm-up while the first slab is in flight ----
            warm = psum.tile([P, 512], F32, tag="ssq")
            for i in range(N_WARMUP):
                nc.tensor.matmul(warm[:, :64], wt8_s[:, 0, :],
                                 wt8_s[:, i % HC, :64], start=True, stop=True)

            state = {}

            def emit_sim(g):
                # raw scores straight off the fp8 slab: 6 accumulation
                # passes per doc, 4 docs concurrent in 32-col groups
                slab = slabs[g]
                lt = lts[g]
                sim = psum.tile([P, 512], F32, tag="sim")
                state[g] = (state[g][0], sim)
                for c in range(HC):
                    for d in range(4):
                        nc.tensor.matmul(
                            sim[32 * d:32 * d + 32, :lt],
                            qw_s[:, c, 32 * d:32 * d + 32],
                            slab[:, d, c, :],
                            start=(c == 0), stop=(c == HC - 1),
                            tile_position=(0, 32 * d))

            def emit_epi(g):
                sq4, sim = state[g]
                lt = lts[g]
                ssq = psum.tile([P, 512], F32, tag="ssq")
                for d in range(4):
                    nc.tensor.matmul(ssq[32 * d:32 * d + 32, :lt],
                                     ones_s, sq4[:, d, :],
                                     start=True, stop=True,
                                     tile_position=(0, 32 * d))
                invb = work.tile([P, lt], F32, tag="invb")
                _rsqrt_act(nc, invb, ssq[:, :lt], eps_t[:, :])
                scaled = work.tile([P, lt], BF16, tag="scaled")
                nc.vector.tensor_mul(scaled, sim[:, :lt], invb)
                nc.vector.reduce_max(out=maxcol[:, g:g + 1], in_=scaled,
                                     axis=mybir.AxisListType.X)

            # ---- doc loop: 16 pairs, epilogue per quad, 1-pair pipelined ----
            for pp in range(DPC // 2):
                g = pp // 2
                lt = lts[g]
                if pp % 2 == 0:
                    if g > 0:
                        slab_g = slabp.tile([P, 4, HC, lt], FP8, tag="slab")
                        slabs[g] = slab_g
                        if g == 1:
                            for d in range(4):
                                nc.sync.dma_start(out=slab_g[:, d],
                                                  in_=dt_d[g][:, d])
                        else:
                            nc.sync.dma_start(out=slab_g, in_=dt_d[g][:])
                    sq4 = work.tile([P, 4, lt], BF16, tag="sq4")
                    state[g] = (sq4, None)
                slab = slabs[g]
                sq4 = state[g][0]
                pd = psum.tile([DIM, 2, 512], F32, tag="pd")
                for c in range(0, HC, 2):
                    for j in range(2):
                        d = 2 * (pp % 2) + j
                        nc.tensor.matmul(pd[:, j, :lt], wt8_s[:, c:c + 2, :],
                                         slab[:, d, c:c + 2, :],
                                         start=(c == 0), stop=(c == HC - 2),
                                         perf_mode=DR)
                pr = pp % 2
                nc.scalar.square(sq4[:, 2 * pr:2 * pr + 2, :], pd[:, :, :lt])
                if pp % 2 == 1:
                    emit_sim(g)
                if pp % 2 == 0 and pp >= 2:
                    emit_epi(g - 1)
            emit_epi(NQUAD - 1)

            # ---- sum over queries + writeback ----
            po = psum.tile([4, NQUAD], F32, tag="sim")
            nc.tensor.matmul(po, blockones, maxcol, start=True, stop=True)
            out_s = work.tile([4, NQUAD], F32, tag="outrow", bufs=1)
            nc.vector.tensor_copy(out_s, po)
            nc.sync.dma_start(out=out_d[:], in_=out_s)
    nc.compile()
    return nc


def _get_nc():
    nc = _NC_CACHE.get(_LTS)
    if nc is None:
        nc = _NC_CACHE[_LTS] = _build_nc(_LTS)
    return nc


def _prep_in_maps(q_hidden, d_hidden, W, d_mask):
    global _LTS, _PERM
    f8 = ml_dtypes.float8_e4m3
    cnt = d_mask.sum(1)
    order = np.argsort(~d_mask, axis=1, kind="stable")
    # quad g on every core = the rank-g doc (by valid count) of each of
    # the core's 4 query batches; position within quad = batch index
    perm = np.zeros((N_CORES, NQUAD, 4), dtype=np.int64)
    for c in range(N_CORES):
        for b in range(4):
            docs = np.arange((4 * c + b) * PPQ, (4 * c + b + 1) * PPQ)
            perm[c, :, b] = docs[np.argsort(-cnt[docs], kind="stable")]
    _PERM = perm
    lts = tuple(int(max(LT_MIN, (int(cnt[perm[:, g, :]].max()) + 15)
                        // 16 * 16)) for g in range(NQUAD))
    _LTS = lts
    wt_t = np.ascontiguousarray(W.T.reshape(HC, P, DIM).transpose(1, 0, 2))
    wt8 = (wt_t * W8SCALE).astype(f8)
    # query side on host: qw = 8 * W.T @ l2norm(W @ q)  [H, 128q] per core
    qf = q_hidden.reshape(-1, H).astype(np.float32)          # [1024q, H]
    qp = qf @ W.T                                            # [1024q, dim]
    qp /= np.maximum(np.sqrt((qp * qp).sum(-1, keepdims=True)), 1e-12)
    qw = (qp @ W) * W8SCALE                                  # [1024q, H]
    in_maps = []
    for c in range(N_CORES):
        m = {"wt8": wt8}
        for g in range(NQUAD):
            lt = lts[g]
            ids = perm[c, g, :]                               # 4 global docs
            idxg = np.where(np.arange(lt)[None, :] >= cnt[ids][:, None],
                            order[ids, :1], order[ids, :lt])
            dcg = np.take_along_axis(d_hidden[ids], idxg[:, :, None], axis=1)
            dtg = dcg.astype(f8).transpose(0, 2, 1)           # [4, 768, lt]
            dtg = dtg.reshape(4, HC, P, lt).transpose(2, 0, 1, 3)
            m[f"dt{g}"] = np.ascontiguousarray(dtg)           # [P, 4, HC, lt]
        qsl = qw[c * QPC:(c + 1) * QPC]                       # [128q, H]
        qm = qsl.T.reshape(HC, P, QPC)                        # [6, 128, 128]
        m["qw"] = np.ascontiguousarray(qm.transpose(1, 0, 2)).astype(f8)
        in_maps.append(m)
    return in_maps


def _run(in_maps, trace=False, **kw):
    res = run_bass_kernel_spmd(
        _get_nc(), in_maps, core_ids=list(range(N_CORES)), trace=trace, **kw)
    # per-core output is [4, NQUAD]: value [b, g] -> doc _PERM[core, g, b]
    out = np.zeros(N_CORES * DPC, dtype=np.float32)
    for c in range(N_CORES):
        r = res.results[c]["out"].astype(np.float32)          # [4, NQUAD]
        for g in range(NQUAD):
            out[_PERM[c, g, :]] = r[:, g]
    return out, res


def kernel(q_hidden, d_hidden, W, d_mask, ppq):
    q_hidden = np.asarray(q_hidden, dtype=np.float32)
    d_hidden = np.asarray(d_hidden, dtype=np.float32)
    W = np.asarray(W, dtype=np.float32)
    d_mask = np.asarray(d_mask).astype(bool)
    in_maps = _prep_in_maps(q_hidden, d_hidden, W, d_mask)
    out, _ = _run(in_maps, trace=False)
    return out


# revision 15
# speedup vs baseline: 1.1132x; 1.1132x over previous
"""ColBERT MaxSim kernel for 8 Trainium2 NeuronCores (Bass/Tile).

Strategy: data-parallel over the 256-doc batch (32 docs per core).

Host side:
  - compacts each doc's VALID tokens (d_mask is ~50% dense) to the
    front and pads to a per-quad budget with a COPY of the doc's first
    valid token.  Duplicating a valid token leaves the per-(query,doc)
    max unchanged, so this is exactly equivalent to -inf masking.
  - VARIABLE per-quad budgets: each core holds 4 query batches x 8
    docs; each batch's docs are sorted by valid count, and quad g takes
    the rank-g doc of every batch (position d <-> batch d, so the
    kernel's doc->query mapping stays compile-time static).  Budget
    lt[g] = max valid count over quad-slot g across all cores (rounded
    to 16), so early quads are ~288 tokens and late ones ~240.
  - computes the query side on host in fp32 and folds it through W:
    qw = W.T @ l2norm(W @ q) [H, 128q], so the device computes raw
    scores DIRECTLY off the fp8 doc stream (contraction over H=768).
  - W and qw are pre-scaled by 8 so fp8(e4m3) entries land in the
    normal range; the scales cancel exactly in sim * rsqrt(|8Wd|^2).

Per core (32 docs = 8 quads):
  per pair of docs (DoubleRow fp8, K=256 per pass):
    pd[:, j] = W8.T @ dT[d]            [128dim, lt] f32 PSUM
    sq = pd^2   (one ACT square per pair, bf16 -> SBUF)
  per quad g (4 docs on col-groups cg = d%4; M=32 matmuls run
  CONCURRENTLY in distinct 32-col groups of the PE array):
    sim[32cg:+32, :] += qw8[:, c, 32cg].T @ dT[d]   (6 passes, fp8)
    ssq[32cg:+32, :]  = ones[:, :32].T @ sq[d]      (per-token sumsq)
    invb = rsqrt(ssq + eps)            (ACT)
    scaled = sim * invb                (DVE, bf16)
    maxcol[:, g] = max_tok(scaled)     (DVE reduce_max)
  out[4, 8] = blockones.T @ maxcol     (sum over 32 queries via matmul)

PE warm-up: the HAM clock gate keeps the PE at 1.2 GHz until ~3.4us of
sustained activity; dummy fp8 matmuls run while the first doc slab is
still in flight so the real work starts at 2.4 GHz.
"""

import numpy as np
import ml_dtypes

import concourse.bass as bass
import concourse.bacc as bacc
import concourse.mybir as mybir
import concourse.tile as tile
from concourse.bass_utils import run_bass_kernel_spmd

N_CORES = 8
H, HC, P = 768, 6, 128   # hidden dim, h-chunks, partitions
DIM = 128                # projection dim
DPC = 32                 # docs per core
QPC = 128                # query vectors per core (4 batches x 32)
PPQ = 8                  # passages per query
NQUAD = DPC // 4
BF16 = mybir.dt.bfloat16
FP8 = mybir.dt.float8e4
F32 = mybir.dt.float32
EPS2 = 1e-12
LT_MIN = 64              # floor on per-quad compacted token budget
W8SCALE = 8.0            # fp8 pre-scale on W / qw; cancels in normalization
N_WARMUP = 24            # dummy PE matmuls to lift the HAM clock gate

_LTS = (288,) * NQUAD
_NC_CACHE = {}


def _rsqrt_act(nc, out, in_, bias_ap):
    """out = 1/sqrt(in_ + bias). Emits the Rsqrt activation directly
    (bass's helper refuses it; the 40k-entry reciprocal_sqrt HW table is
    plenty accurate for this kernel's fp8-dominated error budget)."""
    eng = nc.scalar
    ins = [eng.lower_ap(in_), eng.lower_ap(bias_ap),
           mybir.ImmediateValue(dtype=mybir.dt.float32, value=1.0),
           mybir.ImmediateValue(dtype=mybir.dt.float32, value=0.0)]
    return eng.add_instruction(mybir.InstActivation(
        name=nc.get_next_instruction_name(),
        func=mybir.ActivationFunctionType.Rsqrt,
        ins=ins, outs=[eng.lower_ap(out)]))


def _build_nc(lts):
    nc = bacc.Bacc()
    dt_d = [nc.declare_dram_parameter(f"dt{g}", [P, 4, HC, lts[g]], FP8,
                                      isOutput=False) for g in range(NQUAD)]
    qw_d = nc.declare_dram_parameter("qw", [P, HC, QPC], FP8, isOutput=False)
    wt8_d = nc.declare_dram_parameter("wt8", [P, HC, DIM], FP8, isOutput=False)
    out_d = nc.declare_dram_parameter("out", [4, NQUAD], F32, isOutput=True)
    DR = mybir.MatmulPerfMode.DoubleRow

    with tile.TileContext(nc) as tc:
        with (
            tc.tile_pool(name="const", bufs=1) as const,
            tc.tile_pool(name="slab", bufs=4) as slabp,
            tc.tile_pool(name="work", bufs=2) as work,
            tc.tile_pool(name="psum", bufs=2, space=bass.MemorySpace.PSUM) as psum,
        ):
            # ---- input DMAs, ordered for earliest PE start ----
            wt8_s = const.tile([P, HC, DIM], FP8)
            nc.sync.dma_start(out=wt8_s, in_=wt8_d[:])
            slabs = {}
            slab0 = slabp.tile([P, 4, HC, lts[0]], FP8, tag="slab")
            slabs[0] = slab0
            nc.sync.dma_start(out=slab0[:, 0], in_=dt_d[0][:, 0])
            nc.sync.dma_start(out=slab0[:, 1], in_=dt_d[0][:, 1])
            qw_s = const.tile([P, HC, QPC], FP8)
            nc.sync.dma_start(out=qw_s, in_=qw_d[:])
            nc.sync.dma_start(out=slab0[:, 2], in_=dt_d[0][:, 2])
            nc.sync.dma_start(out=slab0[:, 3], in_=dt_d[0][:, 3])

            # ---- constants ----
            ones_raw = const.tile([P, 32], BF16)
            nc.vector.memset(ones_raw, 1.0)
            ones_s = const.tile([P, 32], BF16)     # all-ones lhsT
            nc.scalar.copy(ones_s, ones_raw)
            blk_raw = const.tile([P, 4], F32)      # block-diag ones: col b = 1
            nc.vector.memset(blk_raw, 0.0)         # on partitions 32b..32b+32
            for b in range(4):
                nc.vector.memset(blk_raw[32 * b:32 * b + 32, b:b + 1], 1.0)
            blockones = const.tile([P, 4], F32)
            nc.scalar.copy(blockones, blk_raw)
            eps_t = const.tile([P, 1], F32)        # rsqrt bias (l2norm eps^2)
            nc.vector.memset(eps_t, EPS2)
            maxcol = const.tile([P, NQUAD], F32)   # [4docs x 32q, quads]

            # ---- PE warm-up while the first slab is in flight ----
            warm = psum.tile([P, 512], F32, tag="ssq")
            for i in range(N_WARMUP):
                nc.tensor.matmul(warm[:, :64], wt8_s[:, 0, :],
                                 wt8_s[:, i % HC, :64], start=True, stop=True)

            state = {}

            def emit_sim(g):
                # raw scores straight off the fp8 slab: 6 accumulation
                # passes per doc, 4 docs concurrent in 32-col groups
                slab = slabs[g]
                lt = lts[g]
                sim = psum.tile([P, 512], F32, tag="sim")
                state[g] = (state[g][0], sim)
                for c in range(HC):
                    for d in range(4):
                        nc.tensor.matmul(
                            sim[32 * d:32 * d + 32, :lt],
                            qw_s[:, c, 32 * d:32 * d + 32],
                            slab[:, d, c, :],
                            start=(c == 0), stop=(c == HC - 1),
                            tile_position=(0, 32 * d))

            def emit_epi(g, halves=1):
                # halves=2 processes docs {0,1} then {2,3} separately to
                # shorten the serial tail chain of the final quad
                sq4, sim = state[g]
                lt = lts[g]
                step = 4 // halves
                for h in range(halves):
                    # each half gets its own PSUM bank: PE writing a bank
                    # while ACT/DVE read it is a fatal HW collision
                    ssq = psum.tile([P, 512], F32, tag="ssq")
                    for d in range(h * step, (h + 1) * step):
                        nc.tensor.matmul(ssq[32 * d:32 * d + 32, :lt],
                                         ones_s, sq4[:, d, :],
                                         start=True, stop=True,
                                         tile_position=(0, 32 * d))
                    r0, r1 = 32 * h * step, 32 * (h + 1) * step
                    invb = work.tile([P, lt], F32, tag="invb")
                    _rsqrt_act(nc, invb[r0:r1], ssq[r0:r1, :lt], eps_t[r0:r1])
                    scaled = work.tile([P, lt], BF16, tag="scaled")
                    nc.vector.tensor_mul(scaled[r0:r1], sim[r0:r1, :lt],
                                         invb[r0:r1])
                    nc.vector.reduce_max(out=maxcol[r0:r1, g:g + 1],
                                         in_=scaled[r0:r1],
                                         axis=mybir.AxisListType.X)

            # ---- doc loop: 16 pairs, epilogue per quad, 1-pair pipelined ----
            for pp in range(DPC // 2):
                g = pp // 2
                lt = lts[g]
                if pp % 2 == 0:
                    if g > 0:
                        slab_g = slabp.tile([P, 4, HC, lt], FP8, tag="slab")
                        slabs[g] = slab_g
                        nc.sync.dma_start(out=slab_g, in_=dt_d[g][:])
                    if pp >= 2:
                        # epilogue of the previous quad fires at the quad
                        # boundary, before this quad's projections, so
                        # jones/rsqrt don't head-block the ACT queue
                        emit_epi(g - 1)
                    sq4 = work.tile([P, 4, lt], BF16, tag="sq4")
                    state[g] = (sq4, None)
                slab = slabs[g]
                sq4 = state[g][0]
                pd = psum.tile([DIM, 2, 512], F32, tag="pd")
                for c in range(0, HC, 2):
                    for j in range(2):
                        d = 2 * (pp % 2) + j
                        nc.tensor.matmul(pd[:, j, :lt], wt8_s[:, c:c + 2, :],
                                         slab[:, d, c:c + 2, :],
                                         start=(c == 0), stop=(c == HC - 2),
                                         perf_mode=DR)
                pr = pp % 2
                nc.scalar.square(sq4[:, 2 * pr:2 * pr + 2, :], pd[:, :, :lt])
                if pp % 2 == 1:
                    emit_sim(g)
            emit_epi(NQUAD - 1, halves=2)

            # ---- sum over queries + writeback ----
            po = psum.tile([4, NQUAD], F32, tag="sim")
            nc.tensor.matmul(po, blockones, maxcol, start=True, stop=True)
            out_s = work.tile([4, NQUAD], F32, tag="outrow", bufs=1)
            nc.vector.tensor_copy(out_s, po)
            nc.sync.dma_start(out=out_d[:], in_=out_s)
    nc.compile()
    return nc


def _get_nc():
    nc = _NC_CACHE.get(_LTS)
    if nc is None:
        nc = _NC_CACHE[_LTS] = _build_nc(_LTS)
    return nc


def _prep_in_maps(q_hidden, d_hidden, W, d_mask):
    global _LTS, _PERM
    f8 = ml_dtypes.float8_e4m3
    cnt = d_mask.sum(1)
    order = np.argsort(~d_mask, axis=1, kind="stable")
    # quad g on every core = the rank-g doc (by valid count) of each of
    # the core's 4 query batches; position within quad = batch index
    perm = np.zeros((N_CORES, NQUAD, 4), dtype=np.int64)
    for c in range(N_CORES):
        for b in range(4):
            docs = np.arange((4 * c + b) * PPQ, (4 * c + b + 1) * PPQ)
            perm[c, :, b] = docs[np.argsort(-cnt[docs], kind="stable")]
    _PERM = perm
    lts = tuple(int(max(LT_MIN, (int(cnt[perm[:, g, :]].max()) + 15)
                        // 16 * 16)) for g in range(NQUAD))
    _LTS = lts
    wt_t = np.ascontiguousarray(W.T.reshape(HC, P, DIM).transpose(1, 0, 2))
    wt8 = (wt_t * W8SCALE).astype(f8)
    # query side on host: qw = 8 * W.T @ l2norm(W @ q)  [H, 128q] per core
    qf = q_hidden.reshape(-1, H).astype(np.float32)          # [1024q, H]
    qp = qf @ W.T                                            # [1024q, dim]
    qp /= np.maximum(np.sqrt((qp * qp).sum(-1, keepdims=True)), 1e-12)
    qw = (qp @ W) * W8SCALE                                  # [1024q, H]
    in_maps = []
    for c in range(N_CORES):
        m = {"wt8": wt8}
        for g in range(NQUAD):
            lt = lts[g]
            ids = perm[c, g, :]                               # 4 global docs
            idxg = np.where(np.arange(lt)[None, :] >= cnt[ids][:, None],
                            order[ids, :1], order[ids, :lt])
            dcg = np.take_along_axis(d_hidden[ids], idxg[:, :, None], axis=1)
            dtg = dcg.astype(f8).transpose(0, 2, 1)           # [4, 768, lt]
            dtg = dtg.reshape(4, HC, P, lt).transpose(2, 0, 1, 3)
            m[f"dt{g}"] = np.ascontiguousarray(dtg)           # [P, 4, HC, lt]
        qsl = qw[c * QPC:(c + 1) * QPC]                       # [128q, H]
        qm = qsl.T.reshape(HC, P, QPC)                        # [6, 128, 128]
        m["qw"] = np.ascontiguousarray(qm.transpose(1, 0, 2)).astype(f8)
        in_maps.append(m)
    return in_maps


def _run(in_maps, trace=False, **kw):
    res = run_bass_kernel_spmd(
        _get_nc(), in_maps, core_ids=list(range(N_CORES)), trace=trace, **kw)
    # per-core output is [4, NQUAD]: value [b, g] -> doc _PERM[core, g, b]
    out = np.zeros(N_CORES * DPC, dtype=np.float32)
    for c in range(N_CORES):
        r = res.results[c]["out"].astype(np.float32)          # [4, NQUAD]
        for g in range(NQUAD):
            out[_PERM[c, g, :]] = r[:, g]
    return out, res


def kernel(q_hidden, d_hidden, W, d_mask, ppq):
    q_hidden = np.asarray(q_hidden, dtype=np.float32)
    d_hidden = np.asarray(d_hidden, dtype=np.float32)
    W = np.asarray(W, dtype=np.float32)
    d_mask = np.asarray(d_mask).astype(bool)
    in_maps = _prep_in_maps(q_hidden, d_hidden, W, d_mask)
    out, _ = _run(in_maps, trace=False)
    return out
